# revision 1
# baseline (speedup 1.0000x reference)
"""Trainium2 Bass kernel for nn_DependencyLSTMLocalModel.

Model: word-embedding gather + masked mean-pool of dependency embeddings
(segment_reduce) + BiLSTM(H=128) over S=512 + max-pool over time + linear
classifier.

Sharding: data-parallel over batch. B=32 across 8 cores -> 4 sequences per
core. Embedding tables + weights replicated. No collectives; host
concatenates the per-core [4, 5] logits.

All shapes hardcoded per the problem spec:
  word_ids [32,3,512] i32, deps_ids [32,512,8] i32,
  word_table [100000,300] f32, dep_table [64,300] f32,
  Wih_* [512,300], Whh_* [512,128], b_* [512], W_cls [5,256], b_cls [5].
"""

import sys

for _p in ("/opt/trn_rl_repo",):
    if _p not in sys.path:
        sys.path.insert(0, _p)

import numpy as np

from concourse import bass, mybir
import concourse.tile as tile
from concourse.tile import add_dep_helper
from concourse.bass import IndirectOffsetOnAxis
from concourse.bass_utils import run_bass_kernel_spmd
from concourse.masks import make_identity

F32 = mybir.dt.float32
I32 = mybir.dt.int32

N_CORES = 8
B = 32          # full batch
BL = B // N_CORES  # batch per core = 4
S = 512         # sequence length
E = 300         # embedding dim
D = 8           # deps per token
H = 128         # LSTM hidden
V_DEP = 64      # dep vocab
NTOK = BL * S   # tokens per core = 2048
NTILE = NTOK // 128  # 16 token tiles per core
EC = [128, 128, 44]  # E=300 split into k-chunks
NG = 4          # gates (reordered i,f,o,g)

# xbuf free-layout: column  s*4*BL + g*BL + b   (per direction)
XW = NG * BL    # 16 columns per step
S_LOOP = S      # dev knob: truncate the LSTM loop for timing decomposition


def _build_program():
    nc = bass.Bass("TRN2", target_bir_lowering=False, debug=False)

    # ---- DRAM inputs (per-core slices / host-prepped weights) ----
    wid = nc.dram_tensor("wid", [NTOK, 1], I32, kind="ExternalInput")
    deps = nc.dram_tensor("deps", [NTOK, D], I32, kind="ExternalInput")
    word_table = nc.dram_tensor("word_table", [100000, E], F32, kind="ExternalInput")
    # dep_table rows 0,1 zeroed, plus count column -> [64, 301]
    dep_rhs = nc.dram_tensor("dep_rhs", [V_DEP, E + 1], F32, kind="ExternalInput")
    # per (dir, gate): Whh_g^T  [2,4,128,128] flattened
    whhT = nc.dram_tensor("whhT", [2 * NG * H, H], F32, kind="ExternalInput")
    # per (dir, gate): Wih_g^T  [2,4,300,128] flattened
    wihT = nc.dram_tensor("wihT", [2 * NG * E, H], F32, kind="ExternalInput")
    # bias columns [128, 2*4]  (col = dir*4 + gate)
    biasT = nc.dram_tensor("biasT", [H, 2 * NG], F32, kind="ExternalInput")
    # classifier: W_cls^T split [256, 5] and bias [1, 5]
    wclsT = nc.dram_tensor("wclsT", [2 * H, 5], F32, kind="ExternalInput")
    bcls = nc.dram_tensor("bcls", [BL, 5], F32, kind="ExternalInput")

    logits = nc.dram_tensor("logits", [BL, 5], F32, kind="ExternalOutput")

    with tile.TileContext(nc) as tc:
        with (
            tc.tile_pool(name="const", bufs=1) as cpool,
            tc.tile_pool(name="work", bufs=3) as wpool,
            tc.tile_pool(name="emb", bufs=1) as epool,
            tc.tile_pool(name="state", bufs=1) as spool,
        ):
            # ---------- constants ----------
            ident = cpool.tile([128, 128], F32)
            make_identity(nc, ident[:])

            iota2d_i = cpool.tile([128, V_DEP], I32)
            nc.gpsimd.iota(iota2d_i[:], pattern=[[1, V_DEP]], base=0,
                           channel_multiplier=0)
            iota2d = cpool.tile([128, V_DEP], F32)
            nc.vector.tensor_copy(out=iota2d[:], in_=iota2d_i[:])
            dep_rhs_sb = cpool.tile([V_DEP, E + 1], F32)
            nc.sync.dma_start(out=dep_rhs_sb[:], in_=dep_rhs[:])
            bias_sb = cpool.tile([H, 2 * NG], F32)
            nc.sync.dma_start(out=bias_sb[:], in_=biasT[:])
            whh_sb = []  # [dir][gate] -> [128,128]
            for d in range(2):
                row = []
                for g in range(NG):
                    t = cpool.tile([H, H], F32, tag=f"whh_{d}_{g}", name=f"whh_{d}_{g}")
                    off = (d * NG + g) * H
                    nc.sync.dma_start(out=t[:], in_=whhT[off:off + H, :])
                    row.append(t)
                whh_sb.append(row)
            wih_sb = []  # [dir][gate][chunk] -> [<=128, 128]
            for d in range(2):
                row = []
                for g in range(NG):
                    chunks = []
                    base = (d * NG + g) * E
                    off = 0
                    for ci, w in enumerate(EC):
                        t = cpool.tile([w, H], F32, tag=f"wih_{d}_{g}_{ci}", name=f"wih_{d}_{g}_{ci}")
                        nc.sync.dma_start(out=t[:], in_=wihT[base + off:base + off + w, :])
                        chunks.append(t)
                        off += w
                    row.append(chunks)
                wih_sb.append(row)
            wcls_f = cpool.tile([H, 5], F32)
            wcls_b = cpool.tile([H, 5], F32)
            nc.sync.dma_start(out=wcls_f[:], in_=wclsT[0:H, :])
            nc.sync.dma_start(out=wcls_b[:], in_=wclsT[H:2 * H, :])
            bcls_sb = cpool.tile([BL, 5], F32)
            nc.sync.dma_start(out=bcls_sb[:], in_=bcls[:])


            # ---------- persistent big buffers ----------
            # x-gates per dir: [128, S*XW]
            xbuf = [epool.tile([H, S * XW], F32, tag=f"xbuf_{d}", name=f"xbuf_{d}") for d in range(2)]
            # h history per dir: [128, S*BL]
            hbuf = [epool.tile([H, S * BL], F32, tag=f"hbuf_{d}", name=f"hbuf_{d}") for d in range(2)]

            # ---------- phase 1: embeddings ----------
            etpool = tc.alloc_tile_pool(name="embT", bufs=1)
            ppool = tc.alloc_tile_pool(name="psum1", bufs=2, space="PSUM")
            pbig = tc.alloc_tile_pool(name="psbig", bufs=2, space="PSUM")
            # transposed blended embeddings, per batch, per E-chunk: [128, S]
            embsT = [[etpool.tile([128, S], F32, tag=f"embsT_{b}_{c}", name=f"embsT_{b}_{c}")
                      for c in range(3)] for b in range(BL)]
            for ti in range(NTILE):
                idx = wpool.tile([128, 1], I32, tag="idx", bufs=16)
                nc.sync.dma_start(out=idx[:], in_=wid[ti * 128:(ti + 1) * 128, :])
                wrows = wpool.tile([128, E], F32, tag="wrows", bufs=16)
                nc.gpsimd.indirect_dma_start(
                    out=wrows[:], out_offset=None,
                    in_=word_table[:],
                    in_offset=IndirectOffsetOnAxis(ap=idx[:, :1], axis=0),
                )
                dep2i = wpool.tile([128, D], I32, tag="dep2i", bufs=16)
                nc.sync.dma_start(
                    out=dep2i[:], in_=deps[ti * 128:(ti + 1) * 128, :])
                dep2 = wpool.tile([128, D], F32, tag="dep2", bufs=16)
                nc.vector.tensor_copy(out=dep2[:], in_=dep2i[:])
                # one-hot [tok, (d, v)] then counts [tok, v]
                oh = wpool.tile([128, D * V_DEP], F32, tag="oh", bufs=2)
                nc.vector.tensor_tensor(
                    out=oh[:].rearrange("t (d v) -> t d v", v=V_DEP),
                    in0=dep2[:, :, None].to_broadcast([128, D, V_DEP]),
                    in1=iota2d[:, None, :].to_broadcast([128, D, V_DEP]),
                    op=mybir.AluOpType.is_equal,
                )
                cmat = wpool.tile([128, V_DEP], F32, tag="cmat")
                nc.vector.tensor_reduce(
                    out=cmat[:],
                    in_=oh[:].rearrange("t (d v) -> t v d", v=V_DEP),
                    axis=mybir.AxisListType.X,
                    op=mybir.AluOpType.add,
                )
                ctp = ppool.tile([V_DEP, 128], F32, space="PSUM", tag="ctp")
                nc.tensor.transpose(out=ctp[:], in_=cmat[:], identity=ident[:])
                ct = wpool.tile([V_DEP, 128], F32, tag="ct")
                nc.vector.tensor_copy(out=ct[:], in_=ctp[:])
                # dep_sum (+count col): [128 tok, 301]
                dps = ppool.tile([128, E + 1], F32, space="PSUM", tag="dps")
                nc.tensor.matmul(out=dps[:], lhsT=ct[:], rhs=dep_rhs_sb[:],
                                 start=True, stop=True)
                # blend coefficients from count column
                cnt = wpool.tile([128, 1], F32, tag="cnt")
                nc.vector.tensor_copy(out=cnt[:], in_=dps[:, E:E + 1])
                cmax = wpool.tile([128, 1], F32, tag="cmax")
                nc.vector.tensor_scalar_max(out=cmax[:], in0=cnt[:], scalar1=1.0)
                rec = wpool.tile([128, 1], F32, tag="rec")
                nc.vector.reciprocal(out=rec[:], in_=cmax[:])
                sel = wpool.tile([128, 1], F32, tag="sel")
                nc.vector.tensor_single_scalar(
                    out=sel[:], in_=cnt[:], scalar=0.0, op=mybir.AluOpType.is_gt)
                acoef = wpool.tile([128, 1], F32, tag="acoef")
                nc.vector.tensor_scalar(
                    out=acoef[:], in0=sel[:], scalar1=-0.5, scalar2=1.0,
                    op0=mybir.AluOpType.mult, op1=mybir.AluOpType.add)
                bcoef = wpool.tile([128, 1], F32, tag="bcoef")
                nc.vector.tensor_scalar(
                    out=bcoef[:], in0=rec[:], scalar1=0.5, scalar2=sel[:],
                    op0=mybir.AluOpType.mult, op1=mybir.AluOpType.mult)
                # blended = wrows*acoef + dep_sum*bcoef
                dscaled = wpool.tile([128, E], F32, tag="dscaled", bufs=2)
                nc.vector.tensor_scalar_mul(
                    out=dscaled[:], in0=dps[:, 0:E], scalar1=bcoef[:])
                blend = wpool.tile([128, E], F32, tag="blend", bufs=2)
                nc.vector.scalar_tensor_tensor(
                    out=blend[:], in0=wrows[:], scalar=acoef[:], in1=dscaled[:],
                    op0=mybir.AluOpType.mult, op1=mybir.AluOpType.add)
                # transpose into embsT chunks
                b_i, srange = ti // 4, (ti % 4) * 128
                off = 0
                for ci, w in enumerate(EC):
                    tps = ppool.tile([128, 128], F32, space="PSUM", tag="tps")
                    nc.tensor.transpose(
                        out=tps[:w, :128], in_=blend[:, off:off + w], identity=ident[:])
                    nc.vector.tensor_copy(
                        out=embsT[b_i][ci][:w, srange:srange + 128],
                        in_=tps[:w, :128])
                    off += w

            # ---------- phase 2: x-gates ----------
            # xbuf[d][h, s*XW + g*BL + b] = (Wih_dg @ emb_{b,s})[h] + bias_dg[h]
            for b_i in range(BL):
                for d in range(2):
                    for g in range(NG):
                        xp = pbig.tile([H, S], F32, space="PSUM", tag="xp")
                        for ci in range(3):
                            w = EC[ci]
                            nc.tensor.matmul(
                                out=xp[:], lhsT=wih_sb[d][g][ci][:w, :],
                                rhs=embsT[b_i][ci][:w, :],
                                start=(ci == 0), stop=(ci == 2))
                        col = g * BL + b_i
                        dst = xbuf[d][:].rearrange(
                            "h (s xw) -> h s xw", xw=XW)[:, :, col:col + 1]
                        nc.vector.tensor_scalar_add(
                            out=dst,
                            in0=xp[:, :, None],
                            scalar1=bias_sb[:, d * NG + g:d * NG + g + 1])

            pbig.release()
            ppool.release()
            etpool.release()

            # ---------- phase 3: BiLSTM ----------
            plstm = tc.alloc_tile_pool(name="plstm", bufs=2, space="PSUM")
            h0 = spool.tile([H, BL], F32)
            nc.vector.memset(h0[:], 0.0)
            c_t = [spool.tile([H, BL], F32, tag=f"c_{d}", name=f"c_{d}") for d in range(2)]
            for d in range(2):
                nc.vector.memset(c_t[d][:], 0.0)

            for t in range(S_LOOP):
                for d in range(2):
                    s_idx = t if d == 0 else S - 1 - t
                    rhs = h0[:] if t == 0 else hbuf[d][:, (t - 1) * BL:t * BL]
                    gp = plstm.tile([H, XW], F32, space="PSUM", tag=f"gp_{d}", name=f"gp_{d}")
                    nc.tensor.matmul(
                        out=gp[:], lhsT=ident[:],
                        rhs=xbuf[d][:, s_idx * XW:(s_idx + 1) * XW],
                        start=True, stop=False)
                    for g in range(NG):
                        nc.tensor.matmul(
                            out=gp[:, g * BL:(g + 1) * BL],
                            lhsT=whh_sb[d][g][:], rhs=rhs,
                            start=False, stop=(g == NG - 1))
                    act = wpool.tile([H, XW], F32, tag=f"act_{d}", name=f"act_{d}")
                    # i,f,o sigmoid; g pre-scaled so sigmoid gives (tanh+1)/2
                    nc.scalar.activation(
                        out=act[:], in_=gp[:],
                        func=mybir.ActivationFunctionType.Sigmoid)
                    # c = f*c + i*(2*sg-1) = f*c + 2*i*sg - i
                    p = wpool.tile([H, BL], F32, tag=f"p_{d}", name=f"p_{d}")
                    nc.vector.tensor_mul(
                        out=p[:], in0=act[:, 0:BL], in1=act[:, 3 * BL:XW])
                    fc = wpool.tile([H, BL], F32, tag=f"fc_{d}", name=f"fc_{d}")
                    nc.gpsimd.tensor_mul(
                        out=fc[:], in0=act[:, BL:2 * BL], in1=c_t[d][:])
                    u = wpool.tile([H, BL], F32, tag=f"u_{d}", name=f"u_{d}")
                    nc.vector.scalar_tensor_tensor(
                        out=u[:], in0=p[:], scalar=2.0, in1=act[:, 0:BL],
                        op0=mybir.AluOpType.mult, op1=mybir.AluOpType.subtract)
                    cn = spool.tile([H, BL], F32, tag=f"cn_{d}_{t % 2}", name=f"cn_{d}_{t % 2}")
                    nc.vector.tensor_add(out=cn[:], in0=fc[:], in1=u[:])
                    c_t[d] = cn
                    # tanh(c) = 2*sigmoid(2c) - 1
                    sc_t = wpool.tile([H, BL], F32, tag=f"sc_{d}", name=f"sc_{d}")
                    nc.scalar.activation(
                        out=sc_t[:], in_=cn[:],
                        func=mybir.ActivationFunctionType.Sigmoid, scale=2.0)
                    q = wpool.tile([H, BL], F32, tag=f"q_{d}", name=f"q_{d}")
                    nc.vector.tensor_mul(
                        out=q[:], in0=act[:, 2 * BL:3 * BL], in1=sc_t[:])
                    nc.vector.scalar_tensor_tensor(
                        out=hbuf[d][:, t * BL:(t + 1) * BL], in0=q[:], scalar=2.0,
                        in1=act[:, 2 * BL:3 * BL],
                        op0=mybir.AluOpType.mult, op1=mybir.AluOpType.subtract)

            # ---------- phase 4: max-pool + classifier ----------
            hmax = []
            for d in range(2):
                cur = hbuf[d]
                width = S * BL
                while width > BL:
                    half = width // 2
                    nxt = wpool.tile([H, half], F32, tag="mx", bufs=3)
                    nc.vector.tensor_max(
                        out=nxt[:], in0=cur[:, 0:half], in1=cur[:, half:width])
                    cur = nxt
                    width = half
                hmax.append(cur)
            lp = plstm.tile([BL, 5], F32, space="PSUM", tag="lp", bufs=1)
            nc.tensor.matmul(out=lp[:], lhsT=hmax[0][:, 0:BL], rhs=wcls_f[:],
                             start=True, stop=False)
            nc.tensor.matmul(out=lp[:], lhsT=hmax[1][:, 0:BL], rhs=wcls_b[:],
                             start=False, stop=True)
            lout = wpool.tile([BL, 5], F32, tag="lout")
            nc.vector.tensor_add(out=lout[:], in0=lp[:], in1=bcls_sb[:])
            nc.sync.dma_start(out=logits[:], in_=lout[:])
            plstm.release()

    return nc


def _legalize_waits(nc, max_waits=1):
    """walrus codegen caps embedded sync-waits per instruction (1 for fp32
    matmul/ACT/memset structs). Hoist excess waits onto wait-only
    EventSemaphore instructions inserted just before, on the same engine.
    Each carrier also bumps a scratch semaphore (CoreSim requires every
    instruction to have an update)."""
    used = set()
    for bb in nc.main_func.blocks:
        for ins in bb.instructions:
            si = getattr(ins, "sync_info", None)
            if si is not None:
                for w in (si.on_wait or []):
                    used.add(w.id)
                for u in (si.on_update or []):
                    used.add(u.id)
    scratch_id = max(used) + 1
    n_id = 0
    for bb in nc.main_func.blocks:
        newl = []
        for ins in bb.instructions:
            si = getattr(ins, "sync_info", None)
            tn = type(ins).__name__
            if (si is not None and si.on_wait is not None
                    and len(si.on_wait) > max_waits
                    and tn not in ("InstEventSemaphore",)):
                waits = list(si.on_wait)
                for w in waits[:-max_waits]:
                    ev = mybir.InstEventSemaphore(
                        name=f"wsplit_{n_id}",
                        engine=ins.engine,
                        sync_info=mybir.SyncInfo(
                            on_wait=[w],
                            on_update=[mybir.SyncUpdate(
                                sync_type="semaphore", id=scratch_id,
                                ant_name="wsplit_scratch",
                                update_mode="sem-inc", update_value=1)]),
                    )
                    n_id += 1
                    newl.append(ev)
                ins.sync_info = mybir.SyncInfo(
                    on_wait=waits[-max_waits:], on_update=si.on_update)
            newl.append(ins)
        bb.instructions[:] = newl


_NC_CACHE = None


def _get_program():
    global _NC_CACHE
    if _NC_CACHE is None:
        _NC_CACHE = _build_program()
        _legalize_waits(_NC_CACHE)
    return _NC_CACHE


def _prep_host(inputs):
    """Host-side weight reshaping (small tensors only) + per-core slicing."""
    word_ids = np.asarray(inputs["word_ids"])
    deps_ids = np.asarray(inputs["deps_ids"])
    word_table = np.ascontiguousarray(np.asarray(inputs["word_table"], dtype=np.float32))
    dep_table = np.asarray(inputs["dep_table"], dtype=np.float32)

    # dep_rhs: rows 0,1 zeroed + count column
    dep_rhs = np.zeros((V_DEP, E + 1), dtype=np.float32)
    dep_rhs[:, :E] = dep_table
    dep_rhs[0, :E] = 0.0
    dep_rhs[1, :E] = 0.0
    dep_rhs[:, E] = 1.0
    dep_rhs[0, E] = 0.0
    dep_rhs[1, E] = 0.0

    # gate reorder i,f,g,o -> i,f,o,g
    perm = [0, 1, 3, 2]

    def gates_of(w):  # [4H, ...] -> list of 4 [H, ...] in new order
        return [w[g * H:(g + 1) * H] for g in perm]

    whhT = np.zeros((2 * NG * H, H), dtype=np.float32)
    wihT = np.zeros((2 * NG * E, H), dtype=np.float32)
    biasT = np.zeros((H, 2 * NG), dtype=np.float32)
    for d, (wih, whh, bb) in enumerate([
        (inputs["Wih_f"], inputs["Whh_f"], inputs["b_f"]),
        (inputs["Wih_b"], inputs["Whh_b"], inputs["b_b"]),
    ]):
        wih = np.asarray(wih, dtype=np.float32)
        whh = np.asarray(whh, dtype=np.float32)
        bb = np.asarray(bb, dtype=np.float32)
        for g, (hg, ig, bg) in enumerate(zip(gates_of(whh), gates_of(wih), gates_of(bb))):
            sc = 2.0 if g == 3 else 1.0  # tanh(x) = 2*sigmoid(2x) - 1
            whhT[(d * NG + g) * H:(d * NG + g + 1) * H] = sc * hg.T
            wihT[(d * NG + g) * E:(d * NG + g + 1) * E] = sc * ig.T
            biasT[:, d * NG + g] = sc * bg

    wclsT = np.ascontiguousarray(np.asarray(inputs["W_cls"], dtype=np.float32).T)  # [256,5]
    bcls = np.tile(np.asarray(inputs["b_cls"], dtype=np.float32).reshape(1, 5),
                   (BL, 1))

    wid_full = np.ascontiguousarray(word_ids[:, 1, :].astype(np.int32))  # [32,512]
    deps_full = np.ascontiguousarray(deps_ids.astype(np.int32))  # [32,512,8]

    in_maps = []
    for c in range(N_CORES):
        sl = slice(c * BL, (c + 1) * BL)
        in_maps.append({
            "wid": wid_full[sl].reshape(NTOK, 1),
            "deps": deps_full[sl].reshape(NTOK, D),
            "word_table": word_table,
            "dep_rhs": dep_rhs,
            "whhT": whhT,
            "wihT": wihT,
            "biasT": biasT,
            "wclsT": wclsT,
            "bcls": bcls,
        })
    return in_maps


def kernel(**inputs):
    nc = _get_program()
    in_maps = _prep_host(inputs)
    res = run_bass_kernel_spmd(nc, in_maps, core_ids=list(range(N_CORES)))
    return np.concatenate([res.results[c]["logits"] for c in range(N_CORES)], axis=0)

